# revision 9
# baseline (speedup 1.0000x reference)
"""Multi-head attention (B=4, S=2048, E=1024, H=16, D=64) on 8 trn2 cores.

Sharding: core c handles batch b=c//2, query rows half=c%2 (1024 rows each).
Each core computes its output rows fully locally (K/V projection for its batch
is duplicated across the 2 cores sharing a batch) -> no collectives.

Per-core pipeline (all matmuls in fp32r = tf32-class, PSUM accumulates fp32):
  T:  transpose xq/xk/xv to feature-major via PE transpose (identity matmul)
  A:  QT = Wq.T @ XqT + bq            [1024f, 1024q]  resident SBUF
  B:  KT = Wk.T @ XkT + bk            [1024f, 2048k]  staged to DRAM
  C:  V  = Xv @ Wv + bv (row-major)   [2048k, 1024f]  staged to DRAM
  D:  per head h: scoresT[k,q] = KT_h.T-slices @ QT_h ; exp on ACT (scale=1/8,
      no max-subtraction: scores ~ N(0,1)); AV with ones-column appended to V
      computes unnormalized outT and softmax sums in one accumulation;
      normalize rows by 1/sum -> outT resident
  E:  out = outT.T-slices @ Wo + bo (row-major) -> DRAM
"""
import sys

sys.path.insert(0, "/opt/trn_rl_repo")

import numpy as np

P = 128
E = 1024
S = 2048
HQ = 1024          # query rows per core
H = 16
D = 64
B = 4

_CACHE = {}


def _build_nc():
    from concourse import bacc, mybir
    from concourse.tile import TileContext

    F32 = mybir.dt.float32
    F32R = mybir.dt.float32r
    AF = mybir.ActivationFunctionType

    nc = bacc.Bacc(None, target_bir_lowering=False, debug=False)

    xq = nc.declare_dram_parameter("xq", [HQ, E], F32R, isOutput=False)
    xk = nc.declare_dram_parameter("xk", [S, E], F32R, isOutput=False)
    xv = nc.declare_dram_parameter("xv", [S, E], F32R, isOutput=False)
    wq = nc.declare_dram_parameter("wq", [E, E], F32R, isOutput=False)
    wk = nc.declare_dram_parameter("wk", [E, E], F32R, isOutput=False)
    wv = nc.declare_dram_parameter("wv", [E, E], F32R, isOutput=False)
    wo = nc.declare_dram_parameter("wo", [E, E], F32R, isOutput=False)
    bqp = nc.declare_dram_parameter("bq", [E], F32R, isOutput=False)
    bkp = nc.declare_dram_parameter("bk", [E], F32R, isOutput=False)
    bvp = nc.declare_dram_parameter("bv", [E], F32R, isOutput=False)
    bop = nc.declare_dram_parameter("bo", [E], F32R, isOutput=False)
    identp = nc.declare_dram_parameter("ident", [P, P], F32R, isOutput=False)
    onesp = nc.declare_dram_parameter("ones", [P, 1], F32R, isOutput=False)
    onesrp = nc.declare_dram_parameter("onesr", [1, 512], F32R, isOutput=False)
    out = nc.declare_dram_parameter("out", [HQ, E], F32, isOutput=True)

    with TileContext(nc) as tc:
        with (
            tc.tile_pool(name="persist", bufs=1) as persist,
            tc.tile_pool(name="qtp", bufs=8) as qtp,
            tc.tile_pool(name="otp", bufs=8) as otp,
            tc.tile_pool(name="ps", bufs=4, space="PSUM") as ps,
            tc.tile_pool(name="ps2", bufs=2, space="PSUM") as ps2,
            tc.tile_pool(name="dram", bufs=1, space="DRAM") as dram,
        ):
            # ---- constants / biases ----
            ident = persist.tile([P, P], F32R, tag="ident", name="ident_sb")
            nc.sync.dma_start(out=ident[:], in_=identp[:])
            ones = persist.tile([P, 1], F32R, tag="ones", name="ones_sb")
            nc.sync.dma_start(out=ones[:], in_=onesp[:])
            bq_sb = persist.tile([1, E], F32R, tag="bq", name="bq_sb")
            nc.sync.dma_start(out=bq_sb[:], in_=bqp[:].rearrange("(a n) -> a n", a=1))
            bk_sb = persist.tile([1, E], F32R, tag="bk", name="bk_sb")
            nc.sync.dma_start(out=bk_sb[:], in_=bkp[:].rearrange("(a n) -> a n", a=1))
            bv_sb = persist.tile([1, E], F32R, tag="bv", name="bv_sb")
            nc.sync.dma_start(out=bv_sb[:], in_=bvp[:].rearrange("(a n) -> a n", a=1))
            bo_sb = persist.tile([1, E], F32R, tag="bo", name="bo_sb")
            nc.sync.dma_start(out=bo_sb[:], in_=bop[:].rearrange("(a n) -> a n", a=1))
            onesr = persist.tile([1, 512], F32R, tag="onesr", name="onesr_sb")
            nc.sync.dma_start(out=onesr[:], in_=onesrp[:])

            QT = [qtp.tile([P, HQ], F32R, tag="qt", name=f"qt{i}") for i in range(8)]
            OT = [otp.tile([P, HQ], F32R, tag="ot", name=f"ot{i}") for i in range(8)]

            kt_stage = dram.tile([E, S], F32R, tag="ktst", name="kt_stage")
            v_stage = dram.tile([S, E], F32R, tag="vst", name="v_stage")

            # ================= phase T + projections =================
            with (
                tc.tile_pool(name="xt", bufs=8) as xtp,
                tc.tile_pool(name="xld", bufs=5) as xldp,
                tc.tile_pool(name="wgt", bufs=8) as wgtp,
                tc.tile_pool(name="pev", bufs=3) as pevp,
            ):
                def transpose_in(xparam, n_rows, label):
                    """DRAM [n_rows, E] row-major -> 8 SBUF tiles [128, n_rows]
                    feature-major (tile f holds features f*128..f*128+127)."""
                    xts = [
                        xtp.tile([P, S], F32R, tag="xt", name=f"xt_{label}{i}")
                        for i in range(8)
                    ]
                    for rg in range(n_rows // 512):
                        xls = []
                        for ri in range(4):
                            xl = xldp.tile([P, E], F32R, tag="xld",
                                           name=f"xl_{label}{rg}_{ri}")
                            r0 = rg * 512 + ri * P
                            nc.sync.dma_start(out=xl[:], in_=xparam[r0:r0 + P, :])
                            xls.append(xl)
                        for fb in range(8):
                            pt = ps.tile([P, 512], F32R, tag="ps",
                                         name=f"pt_{label}{rg}_{fb}")
                            for ri in range(4):
                                nc.tensor.transpose(
                                    pt[:, ri * P:(ri + 1) * P],
                                    xls[ri][:, fb * P:(fb + 1) * P],
                                    ident[:],
                                )
                            nc.vector.tensor_copy(
                                xts[fb][:, rg * 512:(rg + 1) * 512], pt[:]
                            )
                    return xts

                def load_w(wparam, label):
                    ws = []
                    for kc in range(8):
                        t = wgtp.tile([P, E], F32R, tag="wgt",
                                      name=f"w_{label}{kc}")
                        nc.sync.dma_start(out=t[:], in_=wparam[kc * P:(kc + 1) * P, :])
                        ws.append(t)
                    return ws

                # --- Q projection: QT[f, q] ---
                xqT = transpose_in(xq, HQ, "q")
                wqs = load_w(wq, "q")
                for m in range(8):
                    for qh in range(2):
                        pp = ps.tile([P, 512], F32, tag="ps", name=f"pq{m}_{qh}")
                        for kc in range(8):
                            nc.tensor.matmul(
                                pp[:],
                                wqs[kc][:, m * P:(m + 1) * P],
                                xqT[kc][:, qh * 512:(qh + 1) * 512],
                                start=(kc == 0), stop=False,
                            )
                        nc.tensor.matmul(
                            pp[:], bq_sb[0:1, m * P:(m + 1) * P], onesr[:],
                            start=False, stop=True,
                        )
                        nc.vector.tensor_copy(
                            QT[m][:, qh * 512:(qh + 1) * 512], pp[:]
                        )

                # --- K projection: KT[f, k] -> DRAM stage ---
                xkT = transpose_in(xk, S, "k")
                wks = load_w(wk, "k")
                for m in range(8):
                    for nh in range(4):
                        pp = ps.tile([P, 512], F32, tag="ps", name=f"pk{m}_{nh}")
                        for kc in range(8):
                            nc.tensor.matmul(
                                pp[:],
                                wks[kc][:, m * P:(m + 1) * P],
                                xkT[kc][:, nh * 512:(nh + 1) * 512],
                                start=(kc == 0), stop=False,
                            )
                        nc.tensor.matmul(
                            pp[:], bk_sb[0:1, m * P:(m + 1) * P], onesr[:],
                            start=False, stop=True,
                        )
                        ev = pevp.tile([P, 512], F32R, tag="pev", name=f"evk{m}_{nh}")
                        nc.vector.tensor_copy(ev[:], pp[:])
                        nc.sync.dma_start(
                            out=kt_stage[m * P:(m + 1) * P, nh * 512:(nh + 1) * 512],
                            in_=ev[:],
                        )

                # --- V projection (row-major): V[k, f] -> DRAM stage ---
                xvT = transpose_in(xv, S, "v")
                wvs = load_w(wv, "v")
                for mk in range(16):
                    for nh in range(2):
                        pp = ps.tile([P, 512], F32, tag="ps", name=f"pv{mk}_{nh}")
                        for kc in range(8):
                            nc.tensor.matmul(
                                pp[:],
                                xvT[kc][:, mk * P:(mk + 1) * P],
                                wvs[kc][:, nh * 512:(nh + 1) * 512],
                                start=(kc == 0), stop=False,
                            )
                        # + bv broadcast across rows: ones[P].T outer bv-slice
                        nc.tensor.matmul(
                            pp[:], onesr[0:1, 0:P],
                            bv_sb[0:1, nh * 512:(nh + 1) * 512],
                            start=False, stop=True,
                        )
                        ev = pevp.tile([P, 512], F32R, tag="pev", name=f"evv{mk}_{nh}")
                        nc.vector.tensor_copy(ev[:], pp[:])
                        nc.sync.dma_start(
                            out=v_stage[mk * P:(mk + 1) * P, nh * 512:(nh + 1) * 512],
                            in_=ev[:],
                        )

            # ================= phase D: attention =================
            with (
                tc.tile_pool(name="kth", bufs=2) as ktp,
                tc.tile_pool(name="vp", bufs=32) as vpp,
                tc.tile_pool(name="expt", bufs=20) as expp,
                tc.tile_pool(name="rec", bufs=4) as recp,
            ):
                for h in range(16):
                    if h % 2 == 0:
                        # head-pair tile so the per-head 64-row slice sits at the
                        # same partition base as the matching QT slice
                        kt2 = ktp.tile([P, S], F32R, tag="kth", name=f"kt2_{h // 2}")
                        nc.sync.dma_start(
                            out=kt2[:], in_=kt_stage[(h // 2) * P:(h // 2 + 1) * P, :]
                        )
                    vps = []
                    for kc in range(16):
                        vp = vpp.tile([P, D + 1], F32R, tag="vp", name=f"vp{h}_{kc}")
                        nc.sync.dma_start(
                            out=vp[:, 0:D],
                            in_=v_stage[kc * P:(kc + 1) * P, h * D:(h + 1) * D],
                        )
                        nc.sync.dma_start(out=vp[:, D:D + 1], in_=onesp[:])
                        vps.append(vp)
                    qrow = (h % 2) * D
                    exps = []
                    for kc in range(16):
                        pss = ps2.tile([P, E], F32, tag="ps2",
                                       name=f"pss{h}_{kc}")
                        for qh in range(2):
                            nc.tensor.matmul(
                                pss[:, qh * 512:(qh + 1) * 512],
                                kt2[qrow:qrow + D, kc * P:(kc + 1) * P],
                                QT[h // 2][qrow:qrow + D, qh * 512:(qh + 1) * 512],
                                start=True, stop=True,
                            )
                        ex = expp.tile([P, E], F32R, tag="expt",
                                       name=f"ex{h}_{kc}")
                        nc.scalar.activation(ex[:], pss[:], AF.Exp, scale=0.125)
                        exps.append(ex)
                    for qh in range(2):
                        po = ps.tile([D + 1, 512], F32, tag="ps", name=f"po{h}_{qh}")
                        for kc in range(16):
                            nc.tensor.matmul(
                                po[:], vps[kc][:],
                                exps[kc][:, qh * 512:(qh + 1) * 512],
                                start=(kc == 0), stop=(kc == 15),
                            )
                        rec = recp.tile([1, 512], F32R, tag="rec", name=f"rc{h}_{qh}")
                        with nc.allow_low_precision(reason="softmax recip feeds matmul"):
                            nc.vector.reciprocal(rec[:], po[D:D + 1, :])
                        prep = ps.tile([D, 512], F32, tag="ps", name=f"prep{h}_{qh}")
                        nc.tensor.matmul(prep[:], onesr[0:1, 0:D], rec[:],
                                         start=True, stop=True)
                        prep_sb = recp.tile([D, 512], F32, tag="prepsb",
                                            name=f"prepsb{h}_{qh}")
                        nc.vector.tensor_copy(prep_sb[:], prep[:])
                        nc.vector.tensor_mul(
                            OT[h // 2][qrow:qrow + D, qh * 512:(qh + 1) * 512],
                            po[0:D, :], prep_sb[:],
                        )

            # ================= phase E: output projection =================
            with (
                tc.tile_pool(name="wo", bufs=8) as wop,
                tc.tile_pool(name="oev", bufs=4) as oevp,
            ):
                wos = []
                for kc in range(8):
                    t = wop.tile([P, E], F32R, tag="wo", name=f"wo{kc}")
                    nc.sync.dma_start(out=t[:], in_=wo[kc * P:(kc + 1) * P, :])
                    wos.append(t)
                for mq in range(8):
                    for nh in range(2):
                        pp = ps.tile([P, 512], F32, tag="ps", name=f"pout{mq}_{nh}")
                        for kc in range(8):
                            nc.tensor.matmul(
                                pp[:],
                                OT[kc][:, mq * P:(mq + 1) * P],
                                wos[kc][:, nh * 512:(nh + 1) * 512],
                                start=(kc == 0), stop=False,
                            )
                        nc.tensor.matmul(
                            pp[:], onesr[0:1, 0:P],
                            bo_sb[0:1, nh * 512:(nh + 1) * 512],
                            start=False, stop=True,
                        )
                        ev = oevp.tile([P, 512], F32, tag="oev", name=f"oev{mq}_{nh}")
                        nc.vector.tensor_copy(ev[:], pp[:])
                        nc.sync.dma_start(
                            out=out[mq * P:(mq + 1) * P, nh * 512:(nh + 1) * 512],
                            in_=ev[:],
                        )

    nc.compile()
    return nc


def _get_nc():
    if "nc" not in _CACHE:
        _CACHE["nc"] = _build_nc()
    return _CACHE["nc"]


def kernel(query, key, value, Wq, bq, Wk, bk, Wv, bv, Wo, bo, _trace=False):
    from concourse.bass_utils import run_bass_kernel_spmd

    nc = _get_nc()
    query = np.asarray(query, dtype=np.float32)
    key = np.asarray(key, dtype=np.float32)
    value = np.asarray(value, dtype=np.float32)
    common = {
        "wq": np.ascontiguousarray(Wq, dtype=np.float32),
        "wk": np.ascontiguousarray(Wk, dtype=np.float32),
        "wv": np.ascontiguousarray(Wv, dtype=np.float32),
        "wo": np.ascontiguousarray(Wo, dtype=np.float32),
        "bq": np.ascontiguousarray(bq, dtype=np.float32),
        "bk": np.ascontiguousarray(bk, dtype=np.float32),
        "bv": np.ascontiguousarray(bv, dtype=np.float32),
        "bo": np.ascontiguousarray(bo, dtype=np.float32),
        "ident": np.eye(P, dtype=np.float32),
        "ones": np.ones((P, 1), dtype=np.float32),
        "onesr": np.ones((1, 512), dtype=np.float32),
    }
    in_maps = []
    for c in range(8):
        b, half = c // 2, c % 2
        m = dict(common)
        m["xq"] = np.ascontiguousarray(query[b, half * HQ:(half + 1) * HQ, :])
        m["xk"] = np.ascontiguousarray(key[b])
        m["xv"] = np.ascontiguousarray(value[b])
        in_maps.append(m)

    res = run_bass_kernel_spmd(nc, in_maps, list(range(8)), trace=_trace)
    _CACHE["last_result"] = res
    outp = np.empty((B, 2048, E), dtype=np.float32)
    for c in range(8):
        b, half = c // 2, c % 2
        outp[b, half * HQ:(half + 1) * HQ, :] = res.results[c]["out"]
    return outp
